# revision 1
# baseline (speedup 1.0000x reference)
"""Trainium2 Bass kernel for the CP-PINN tensor reconstruction problem.

Computes, for xs (3,320,1) and three per-axis MLP weight stacks:
    f_d = MLP_d(xs[d])            (320, 64)   [tanh MLP: 1->128->128->128->64]
    out[a,b,c] = sum_r f_0[a,r] * f_1[b,r] * f_2[c,r]   ->  (320, 320, 320) f32

Strategy: data-parallel over the output's first axis across 8 NeuronCores
(40 a-points per core, no collectives). Each core:
  - loads ALL weights/biases with a single host-packed DMA (one 870 KB
    transfer, hoisted out of the benchmark loop along with a one-time
    f32 -> float32r conversion copy of the weight block),
  - computes the three MLPs in transposed (rank-major) layout with
    float32r matmuls (1 cycle/row at N>=256 vs 4 for plain fp32),
    interleaved layer-by-layer across dims on TensorE + ScalarE (tanh);
    the final layer writes each factor into BOTH partition halves (via a
    host-duplicated [w3|w3] weight block — fp32r matmuls may not target
    a col-group offset).  For f0 the halves hold DIFFERENT a-points
    (a in [0,20) low, [20,40) high) so KR ops use all 128 partitions,
  - builds the Khatri-Rao product kr[r, a*320+b] = f0[r,a]*f1[r,b] in
    fp16 on VectorE (tensor_scalar_mul in the 4x 16-bit perf mode,
    ~116 ns per a-point), emitted just-in-time ahead of the consuming
    matmuls,
  - reconstructs its (40*320, 320) output slab with 100 K=64 fp16
    matmuls kr_chunk(64,128)^T @ f2(64,320) — per iteration the 2
    low-half chunks and 2 high-half chunks go to SEPARATE 2-bank PSUM
    pair-tiles (2 tags x bufs=2 = all 8 banks; lo/hi matmuls concurrent
    on PE row groups 0/64).  The low-pair fp16-downcasting copy starts
    right after the 2nd matmul and overlaps the high matmuls (separate
    tile -> no tile-granular false dep), engines alternating
    VectorE/ScalarE per iteration; ring-release latency is one
    pair-copy (~1.1 us) instead of a 4-row quad (~1.9 us).  Output
    streams to HBM as fp16 (halving the HBM-write floor from ~46 us to
    ~23 us/core) via two 164 KB DMAs per iteration on the two HWDGE
    rings (SP = low half, ACT = high half), with 12-deep staging so the
    ~2 us per-DMA completion receipt never stalls reuse.  The host
    upcasts fp16 -> f32 (CP values are O(1); fp16 rounding dominates
    the 6e-4 rel_l2, ~30x inside the 2e-2 gate).

Measured (loop-slope, 8 cores SPMD): ~43-46 us vs the 62.8 us fp32
baseline on the same harness (74.3 us as originally quoted).  Remaining
walls: fp16 DMA stream ~27-29 us (640 B HBM runs), PSUM pair-copy
release chained against the 2-deep PSUM rings, ~7 us MLP head.
"""

import sys

if "/opt/trn_rl_repo" not in sys.path:
    sys.path.insert(0, "/opt/trn_rl_repo")

import numpy as np

import concourse.bacc as bacc
import concourse.mybir as mybir
from concourse import tile
from concourse.bass_utils import run_bass_kernel_spmd

DIMS = 3
N = 320          # points per coordinate axis
R = 64           # CP rank
H = 128          # hidden width
NCORES = 8
NA = N // NCORES          # a-points per core (40)
NROWS = NA * N            # output rows per core (12800)
MCH = 128                 # (a,b)-rows per matmul chunk
NCHUNK = NROWS // MCH     # 100
NPAIR = NCHUNK // 2       # 50 low/high chunks per row-half
NITER = NPAIR // 2        # 25 pair-iterations (2 lo + 2 hi chunks each)
PSB = 512                 # f32 elements per PSUM bank

# Packed-weights column layout (one (128, WCOLS) f32 tensor):
#   [0,384)    w1 (3 x 128 cols)        [384,768)  w2
#   [768,960)  w3 (3 x 64 cols)
#   [960,963) b0  [963,966) b1  [966,969) b2  [969,972) b3 (dup both halves)
#   [972,1356) w0 (row 0 only, 3 x 128 cols)
#   [1356,1740) w3 duplicated [w3|w3] (3 x 128 cols) — lets the d=1/2 final
#   layers write both partition halves with ONE full-col-group fp32r matmul
#   (fp32r + col-group offset 64 fails the walrus ISA check).
W1_OFF, W2_OFF, W3_OFF = 0, 384, 768
B0_OFF, B1_OFF, B2_OFF, B3_OFF = 960, 963, 966, 969
W0_OFF, W3D_OFF, WCOLS = 972, 1356, 1740
# Packed-x layout: (1, 680) = x0(40) | x1(320) | x2(320)
X0_OFF, X1_OFF, X2_OFF, XCOLS = 0, NA, NA + N, NA + 2 * N

F32 = mybir.dt.float32
F32R = mybir.dt.float32r
F16 = mybir.dt.float16
TANH = mybir.ActivationFunctionType.Tanh
IDENT = mybir.ActivationFunctionType.Identity

_PROG = None


def _build_program(loop=1, variant="full"):
    """loop>1 wraps the whole compute body in a Tile hardware For_i that
    repeats it `loop` times inside one NEFF launch — benchmarking only."""
    nc = bacc.Bacc("TRN2", target_bir_lowering=False)

    xp = nc.dram_tensor("xp", [1, XCOLS], F32, kind="ExternalInput")
    wp = nc.dram_tensor("wp", [H, WCOLS], F32, kind="ExternalInput")
    out = nc.dram_tensor("out", [NROWS, N], F16, kind="ExternalOutput")

    with tile.TileContext(nc) as tc:
        with (
            tc.tile_pool(name="consts", bufs=1) as consts,
            tc.tile_pool(name="work", bufs=2) as work,
            tc.tile_pool(name="stage", bufs=4) as stagep,
            tc.tile_pool(name="cp_ps", bufs=2, space="PSUM") as cp_ps,
        ):
            wp_sb = consts.tile([H, WCOLS], F32)
            nc.sync.dma_start(wp_sb[:], wp[:, :])
            # fp32r copy of the weights for the PE (float32r matmuls run at
            # 1 cycle/row vs 4 for fp32); biases stay read from the f32 copy.
            wp_r = consts.tile([H, WCOLS], F32R)
            nc.vector.tensor_copy(wp_r[:], wp_sb[:])

            import contextlib
            loop_cm = (tc.For_i(0, loop, 1,
                                hint_engines=(mybir.EngineType.PE,))
                       if loop > 1 else contextlib.nullcontext())
            with loop_cm:
                _emit_body(nc, tc, consts, work, stagep, cp_ps,
                           xp, out, wp_sb, wp_r, variant)

    nc.compile()
    return nc


def _emit_body(nc, tc, consts, work, stagep, cp_ps,
               xp, out, wp_sb, wp_r, variant="full"):
    outv = out[:, :].rearrange("(m p) c -> p m c", p=MCH)

    def psum_quad(name):
        # 4-bank (8 KB/partition) PSUM tile (copy probes only)
        return cp_ps.tile([MCH, 4 * PSB], F32, name=name, tag="cps_q")

    mlp_tag = [0]

    def psum_pair(name):
        # 2-bank PSUM tile; two tags x bufs=2 = all 8 banks
        mlp_tag[0] ^= 1
        return cp_ps.tile([MCH, 2 * PSB], F32, name=name,
                          tag="cps_a" if mlp_tag[0] else "cps_b")

    if variant == "empty":
        # calibrates the For_i back-edge + fixed per-iteration overhead
        z = work.tile([1, 1], F32, name="z", tag="z")
        nc.vector.memset(z[:], 0.0)
        return

    if variant == "xp_only":
        z = work.tile([1, 1], F32, name="z", tag="z")
        nc.vector.memset(z[:], 0.0)
        nc.scalar.activation(z[:], z[:], TANH)
        xo_sb = work.tile([1, XCOLS], F32, name="xp_sb", tag="xp_sb")
        nc.sync.dma_start(xo_sb[:], xp[:, :])
        xo_r = work.tile([1, XCOLS], F32R, name="xp_r", tag="xp_r")
        nc.vector.tensor_copy(xo_r[:], xo_sb[:])
        return

    if variant.startswith("copyprobe"):
        # PSUM->SBUF fp16 copy-shape probe, 100 chunks total on ONE engine.
        # copyprobe{v,s}{1,2,4,4c}: v=DVE s=ACT; 1=single-chunk, 2=pair,
        # 4=quad 4-row strided, 4c=quad contiguous (copies bank garbage too)
        spec = variant[9:]
        eng = nc.vector.tensor_copy if spec[0] == "v" else nc.scalar.copy
        shape = spec[1:]
        nb = 4 if shape.startswith("4") else 2
        srcp = cp_ps.tile([MCH, nb * PSB], F32, name="cpsrc", tag="cps_q")
        nc.vector.memset(srcp[:, 0:1], 0.0)
        for it in range(NCHUNK // nb):
            stg = stagep.tile([MCH, nb * N], F16, name="stg_lo", tag="stg_lo")
            if shape == "1":
                for k in range(nb):
                    eng(stg[:, k * N:(k + 1) * N], srcp[:, k * PSB:k * PSB + N])
            elif shape == "4c":
                stg2 = stagep.tile([MCH, nb * PSB], F16, name="stg2",
                                   tag="stg2")
                eng(stg2[:, :], srcp[:, :])
            else:
                eng(stg[:, :].rearrange("p (m c) -> p m c", c=N),
                    srcp[:, :].rearrange("p (m k) -> p m k", k=PSB)[:, :, 0:N])
        return

    if variant.startswith("dma_g"):
        # pure output-DMA stream with `g` chunk-pairs per DMA, `b` bufs;
        # trailing 's' = issue ALL DMAs on the SP ring (single-ring test)
        spec = variant[5:]
        sponly = spec.endswith("s")
        swdge = spec.endswith("p")
        if sponly or swdge:
            spec = spec[:-1]
        g, b = (spec.split("b") + ["2"])[:2]
        g, b = int(g), int(b)
        for it in range(0, NITER, g):
            gsz = min(g, NITER - it) * 2          # chunks in this DMA
            t = 2 * it
            stg_lo = stagep.tile([MCH, 2 * g * N], F16, name="stg_lo",
                                 tag="stg_lo", bufs=b)
            stg_hi = stagep.tile([MCH, 2 * g * N], F16, name="stg_hi",
                                 tag="stg_hi", bufs=b)
            nc.vector.memset(stg_lo[:, 0:1], 1.0)
            nc.vector.memset(stg_hi[:, 0:1], 1.0)
            ring2 = (nc.sync if sponly
                     else nc.gpsimd if swdge else nc.scalar)
            nc.sync.dma_start(
                outv[:, t:t + gsz, :],
                stg_lo[:, 0:gsz * N].rearrange("p (m c) -> p m c", c=N))
            ring2.dma_start(
                outv[:, NPAIR + t:NPAIR + t + gsz, :],
                stg_hi[:, 0:gsz * N].rearrange("p (m c) -> p m c", c=N))
        return

    # Factor matrices in rank-major layout across both partition halves.
    # f1/f2: halves are duplicates.  f0: low half holds a in [0,20),
    # high half a in [20,40) -> KR ops engage all 128 partitions.
    # f1/f2 are fp16: the KR ops then run in the DVE 4x perf mode (16-bit
    # in/out) and the CP matmuls in fp16 (1 cycle/row, same as fp32r).
    f0_sb = consts.tile([2 * R, NA // 2], F32)
    f1_sb = consts.tile([2 * R, N], F16)
    f2_sb = consts.tile([2 * R, N], F16)

    warm = work.tile([1, 1], F32, name="warm", tag="warm")
    nc.vector.memset(warm[:], 0.0)
    nc.scalar.activation(warm[:], warm[:], TANH)

    xp_sb = work.tile([1, XCOLS], F32, name="xp_sb", tag="xp_sb")
    nc.sync.dma_start(xp_sb[:], xp[:, :])

    # The three MLPs interleaved layer-by-layer so PE never waits on the
    # ScalarEngine tanh of the same dim (PE executes in program order).
    # Layer 0 runs in plain f32 (xp comes straight off the DMA; f32r
    # would need a serial conversion first); layers 1+ are fp32r.
    # One quad-tile per matmul: Tile tracks deps at TILE granularity, so
    # sharing a tile between matmuls serializes the MLP (measured +8us).
    dims = [(0, X0_OFF, NA), (1, X1_OFF, N), (2, X2_OFF, N)]
    h_cur = {d: xp_sb[:, xoff:xoff + npts] for d, xoff, npts in dims}
    w_l0 = wp_sb[0:1, :]
    for li, (w_off, b_off, w_ap, wid) in enumerate((
            (W0_OFF, B0_OFF, w_l0, H), (W1_OFF, B1_OFF, wp_r, H),
            (W2_OFF, B2_OFF, wp_r, H))):
        for d, _, npts in dims:
            ps = psum_pair(f"ps{li}_{d}")
            nc.tensor.matmul(
                ps[:, 0:npts],
                w_ap[:, w_off + d * wid:w_off + (d + 1) * wid],
                h_cur[d], start=True, stop=True)
            # layer-2 d=0 output feeds the (plain-f32) d=0 final matmuls
            hdt, htag = ((F32, "h2_0") if (li == 2 and d == 0)
                         else (F32R, f"h_{d}"))
            h = work.tile([H, npts], hdt, name=f"h{li}_{d}", tag=htag)
            nc.scalar.activation(h[:], ps[:, 0:npts], TANH,
                                 bias=wp_sb[:, b_off + d:b_off + d + 1])
            h_cur[d] = h
    # Final layer.  d=1/2: one fp32r matmul with duplicated [w3|w3] weights
    # writes both partition halves at once.  d=0: the halves need DIFFERENT
    # a-ranges, so two plain-f32 matmuls (N=20, cheap) via col-group tiling.
    for d, _, npts in dims:
        f_sb, cols = ((f0_sb, NA // 2) if d == 0
                      else (f1_sb, N) if d == 1 else (f2_sb, N))
        ps = psum_pair(f"psf_{d}")
        if d == 0:
            w3 = wp_sb[:, W3_OFF:W3_OFF + R]
            nc.tensor.matmul(ps[0:R, 0:cols], w3, h_cur[0][:, 0:NA // 2],
                             start=True, stop=True, tile_position=(0, 0))
            nc.tensor.matmul(ps[R:2 * R, 0:cols], w3,
                             h_cur[0][:, NA // 2:NA],
                             start=True, stop=True, tile_position=(0, R))
        else:
            w3d = wp_r[:, W3D_OFF + d * H:W3D_OFF + (d + 1) * H]
            nc.tensor.matmul(ps[:, 0:cols], w3d, h_cur[d],
                             start=True, stop=True)
        nc.scalar.activation(f_sb[:], ps[:, 0:cols], IDENT,
                             bias=wp_sb[:, B3_OFF + d:B3_OFF + d + 1])

    if variant == "mlp_only":
        sink = work.tile([2 * R, N], F32, name="sink", tag="sink")
        nc.vector.tensor_copy(sink[:], f2_sb[:])
        nc.vector.tensor_copy(sink[:], f1_sb[:])
        nc.vector.tensor_copy(sink[:, 0:NA // 2], f0_sb[:])
        return

    # Khatri-Rao on VectorE: kr[p, j*N + b] = f1[p, b] * f0[p, j]
    # (one op covers a=j in the low half and a=j+20 in the high half).
    # Emitted just-in-time per pair-iteration so the first CP matmuls
    # aren't delayed by the whole KR phase.
    kr_sb = consts.tile([2 * R, (NA // 2) * N], F16)
    kr_emitted = 0

    def emit_kr_upto(a_need):
        nonlocal kr_emitted
        while kr_emitted < min(a_need, NA // 2):
            j = kr_emitted
            nc.vector.tensor_scalar_mul(
                kr_sb[:, j * N:(j + 1) * N], f1_sb[:, :], f0_sb[:, j:j + 1])
            kr_emitted += 1

    if variant == "mlp_kr":
        emit_kr_upto(NA // 2)
        return

    # CP reconstruction: 25 iterations x 4 chunks (low t, t+1; high t, t+1).
    # Low chunks cover global rows [0, NROWS/2); high chunks the rest.
    # All 4 matmuls of an iteration land in ONE 4-bank PSUM tile — banks
    # 0,1 = low pair, 2,3 = high pair; the lo/hi matmuls run concurrently
    # on PE row groups 0/64.  A single 4-row fp16-casting quad-copy
    # (VectorE on even iterations, ScalarE on odd) moves them into an
    # 8-chunk staging tile shared by 2 iterations, laid out as
    # [lo0..lo3 | hi0..hi3].  Every 2nd iteration, two 328 KB DMAs (the
    # measured knee) stream the group: low half on the SP HWDGE ring,
    # high half on the ACT ring.
    for it in range(NITER):
        t = 2 * it
        # KR coverage for this iteration's chunks plus one iteration ahead
        emit_kr_upto(-(-((t + 4) * MCH) // N))
        # Low pair and high pair in SEPARATE 2-bank tiles: the low-pair
        # copy starts right after the 2nd matmul and overlaps the high
        # matmuls (different tile -> no tile-granular false dependency),
        # and the ring-release latency is a pair-copy, not a quad.
        ps_lo = psum_pair("cps_lo")
        ps_hi = psum_pair("cps_hi")
        eng_a, eng_b = ((nc.vector.tensor_copy, nc.scalar.copy)
                        if it % 2 == 0
                        else (nc.scalar.copy, nc.vector.tensor_copy))
        # SEPARATE lo/hi staging tiles: Tile tracks DMA-read deps at tile
        # granularity, so with one shared tile the low DMA would wait for
        # the HIGH copy too — splitting lets it fire a pair-copy earlier.
        stg_lo = stagep.tile([MCH, 2 * N], F16, name="stg_lo", tag="stg_l",
                             bufs=16)
        stg_hi = stagep.tile([MCH, 2 * N], F16, name="stg_hi", tag="stg_h",
                             bufs=16)
        for k in range(2):
            c0 = (t + k) * MCH
            nc.tensor.matmul(ps_lo[:, k * PSB:k * PSB + N],
                             kr_sb[0:R, c0:c0 + MCH], f2_sb[0:R, :],
                             start=True, stop=True)
        if variant != "no_copy":
            eng_a(stg_lo[:, :].rearrange("p (m c) -> p m c", c=N),
                  ps_lo[:, :].rearrange("p (m k) -> p m k", k=PSB)[:, :, 0:N])
            if variant != "no_dma":
                nc.sync.dma_start(
                    outv[:, t:t + 2, :],
                    stg_lo[:, :].rearrange("p (m c) -> p m c", c=N))
        for k in range(2):
            c0 = (t + k) * MCH
            nc.tensor.matmul(ps_hi[:, k * PSB:k * PSB + N],
                             kr_sb[R:2 * R, c0:c0 + MCH], f2_sb[R:2 * R, :],
                             start=True, stop=True)
        if variant == "no_copy":
            continue
        eng_b(stg_hi[:, :].rearrange("p (m c) -> p m c", c=N),
              ps_hi[:, :].rearrange("p (m k) -> p m k", k=PSB)[:, :, 0:N])
        if variant == "no_dma":
            continue
        nc.scalar.dma_start(
            outv[:, NPAIR + t:NPAIR + t + 2, :],
            stg_hi[:, :].rearrange("p (m c) -> p m c", c=N))


def _get_program():
    global _PROG
    if _PROG is None:
        _PROG = _build_program()
    return _PROG


def _pack_weights(W0, b0, W1, b1, W2, b2, W3, b3):
    wp = np.zeros((H, WCOLS), np.float32)
    for d in range(DIMS):
        wp[:, W1_OFF + d * H:W1_OFF + (d + 1) * H] = W1[d]
        wp[:, W2_OFF + d * H:W2_OFF + (d + 1) * H] = W2[d]
        wp[:, W3_OFF + d * R:W3_OFF + (d + 1) * R] = W3[d]
        wp[:, B0_OFF + d] = b0[d]
        wp[:, B1_OFF + d] = b1[d]
        wp[:, B2_OFF + d] = b2[d]
        wp[0:R, B3_OFF + d] = b3[d]
        wp[R:2 * R, B3_OFF + d] = b3[d]
        wp[0, W0_OFF + d * H:W0_OFF + (d + 1) * H] = W0[d, 0]
        wp[:, W3D_OFF + d * H:W3D_OFF + d * H + R] = W3[d]
        wp[:, W3D_OFF + d * H + R:W3D_OFF + (d + 1) * H] = W3[d]
    return wp


def _make_in_maps(xs, W0, b0, W1, b1, W2, b2, W3, b3):
    f = lambda x: np.ascontiguousarray(np.asarray(x), dtype=np.float32)
    xs = f(xs)
    wp = _pack_weights(f(W0), f(b0), f(W1), f(b1), f(W2), f(b2), f(W3), f(b3))
    in_maps = []
    for i in range(NCORES):
        x = np.empty((1, XCOLS), np.float32)
        x[0, X0_OFF:X0_OFF + NA] = xs[0, i * NA:(i + 1) * NA, 0]
        x[0, X1_OFF:X1_OFF + N] = xs[1, :, 0]
        x[0, X2_OFF:X2_OFF + N] = xs[2, :, 0]
        in_maps.append({"xp": x, "wp": wp})
    return in_maps


def run_spmd(inputs_kwargs, **run_kwargs):
    """Build (cached) program, run on all 8 cores; returns BassKernelResults."""
    nc = _get_program()
    in_maps = _make_in_maps(**inputs_kwargs)
    return run_bass_kernel_spmd(nc, in_maps, core_ids=list(range(NCORES)),
                                **run_kwargs)


def kernel(xs, W0, b0, W1, b1, W2, b2, W3, b3):
    res = run_spmd(dict(xs=xs, W0=W0, b0=b0, W1=W1, b1=b1,
                        W2=W2, b2=b2, W3=W3, b3=b3))
    slabs = [r["out"].astype(np.float32).reshape(NA, N, N)
             for r in res.results]
    return np.concatenate(slabs, axis=0)

